# revision 46
# baseline (speedup 1.0000x reference)
"""Trainium2 Bass kernel for AttentionLayer: out = softmax(relu(xWq+bq) @ relu(xWk+bk)^T) @ x.

Sharding: data-parallel over batch B=8 across the 8 NeuronCores; Q/K weights
replicated. Each core computes one full [2048, 256] attention independently.

Per-core algorithm (S=2048, D=256, F=128), ~65.5us HW (baseline was 75.6):
  - Host supplies BOTH layouts of x, so the device does zero transposes:
      xt [NCH, 128, 2, 512] fp16 -> Q/K projections (contract over d); the
        [p][h][s] chunk packing makes every partition line one contiguous
        2KB DMA packet (DMA cost is per-packet, not per-byte)
      xa [128, 16*258] bf16 -> x tiles [128, 258] with a [1.0, 0.0] column
        pad (the ones column yields softmax row sums through the output
        matmul for free); 8KB partition lines
  - DMA engines round-robin packets across ALL pending transfers (no FIFO
    between transfers), so arrival order is enforced by gating each issue:
    warm-up junk matmuls "read" the destination tiles first (WAR), and the
    2-buffer xT pool holds chunks c2/c3 behind the projections.
  - qT/kT = relu(W^T @ xT + b) in [f=128, s=2048] layout (fp16 matmuls with
    FWL; relu+bias on DVE via tensor_scalar(add,max) so ACT does exp only).
  - S^T[k, q] = kT^T @ qT per 512-wide q chunk (fp32r, N=512); softmax uses
    a fixed shift exp(s - 60) (scores lie in [2, 94]) written straight to
    bf16 by ACT (a tiny dummy activation up front pre-loads the exp table).
  - O_aug[q, 0:258] = sum_k P^T[:,q]^T @ x_aug[k] in bf16 (FWL hides the
    128-col LDWEIGHTS under the N=258 matmul: 112ns/MM sustained);
    O = O_aug[:, :256] * (1 / O_aug[:, 256]).
  - PE program interleaves scores(c) pairs (front-loaded 3-deep = the psA
    window) with out(c-1) q-tiles; Tile's scheduler then trickles out-tile
    matmuls per-k as each exp lands, keeping the PE gapless start to end.
  - fp16 proj path + bf16 P/V keep error ~4.6e-3 of absmax (budget 2e-2).
"""

import sys
import types
from contextlib import ExitStack

import numpy as np

B, S, D, F = 8, 2048, 256, 128
DA = D + 2           # x padded with [ones, zero] columns
SHIFT = 60.0          # fixed softmax shift; scores lie in [2, 94]
QC = 512              # q-chunk width for the scores/exp/output pipeline
NKT = S // 128        # 16 sequence tiles
NCH = S // QC         # 4 q chunks

_cache = {}


def _ntff_hook_shim():
    """The image's antenv lacks axon_hooks; reconstruct the NTFF profile hook
    so run_bass_kernel_spmd(trace=True) works. Harmless if it fails."""
    if "antenv.axon_hooks" in sys.modules:
        return
    try:
        from trn_agent_boot.trn_boot import _ntff_profile_via_ctypes
        hook = _ntff_profile_via_ctypes("/opt/axon/libaxon_pjrt.so")
        mod = types.ModuleType("antenv.axon_hooks")
        mod.get_axon_ntff_profile_hook = lambda: hook
        mod.set_axon_ntff_profile_hook = lambda h: None
        sys.modules["antenv.axon_hooks"] = mod
    except Exception:
        pass


def _build():
    import concourse.bacc as bacc
    import concourse.tile as tile
    from concourse import mybir

    f32 = mybir.dt.float32
    f32r = mybir.dt.float32r
    f16 = mybir.dt.float16
    bf16 = mybir.dt.bfloat16
    Exp = mybir.ActivationFunctionType.Exp
    Add = mybir.AluOpType.add
    Max = mybir.AluOpType.max

    nc = bacc.Bacc("TRN2", target_bir_lowering=False, debug=False)
    xt_d = nc.dram_tensor("xt", [NCH, 128, 2, QC], f16, kind="ExternalInput").ap()
    xa_d = nc.dram_tensor("xa", [128, NKT * DA], bf16, kind="ExternalInput").ap()
    # wq halves | wk halves | bq | bk packed as one [128, 4*128+4] fp16 blob
    # (biases are fp32 pairs bitcast into the last 4 fp16 columns) so the
    # whole weight load is one DMA with 1KB contiguous partition lines
    wp_d = nc.dram_tensor("wp", [128, 4 * F + 4], f16, kind="ExternalInput").ap()
    out_d = nc.dram_tensor("out", [S, D], f32, kind="ExternalOutput").ap()

    with tile.TileContext(nc) as tc:
        with ExitStack() as ctx:
            cons = ctx.enter_context(tc.tile_pool(name="cons", bufs=1))
            ptp = ctx.enter_context(tc.tile_pool(name="ptp", bufs=2))
            scl = ctx.enter_context(tc.tile_pool(name="scl", bufs=4))
            psA = ctx.enter_context(tc.tile_pool(name="psA", bufs=3, space="PSUM"))
            psB = ctx.enter_context(tc.tile_pool(name="psB", bufs=2, space="PSUM"))

            # ---- input DMAs + PE warm-up, interlocked --------------------
            # The DMA engines round-robin packets across every pending
            # transfer (no FIFO priority between transfers), so concurrent
            # transfers all finish late together. To stream the xT chunks in
            # arrival order, each transfer's issue is GATED on a warm-up junk
            # matmul that "reads" the destination tile before the DMA writes
            # it (WAR dependency): dma(c1) waits w1, dma(xa) waits w2, and
            # the chunk pool's 2 buffers gate c2/c3 on the projections.
            # The junk matmuls double as the HAM warm-up (~0.43us each cold).
            # Garbage operand values are fine: warm-up results are never
            # read and later scores overwrite the PSUM banks.
            xtp = ctx.enter_context(tc.tile_pool(name="xtp", bufs=2))
            wpk = cons.tile([128, 4 * F + 4], f16, tag="wpk")
            wq = [wpk[:, h * F:(h + 1) * F] for h in range(2)]
            wk = [wpk[:, (2 + h) * F:(3 + h) * F] for h in range(2)]
            bq_t = wpk[:, 4 * F:4 * F + 2].bitcast(f32)
            bk_t = wpk[:, 4 * F + 2:4 * F + 4].bitcast(f32)
            biasC = cons.tile([128, 1], f32, tag="biasC")
            nc.gpsimd.memset(biasC[:], -SHIFT)
            xab = cons.tile([128, NKT, DA], bf16, tag="xab")
            x_aug = [xab[:, kt, :] for kt in range(NKT)]

            # dummy activation right away: pulls ACT_TABLE_LOAD (1.3us) off
            # the first-exp critical path (it otherwise waits on the first
            # score matmuls through the scheduler's ordering)
            dum = cons.tile([128, 2], f32, tag="dum")
            nc.scalar.activation(dum[:, 0:1], wpk[:, 0:2].bitcast(f32),
                                 Exp, bias=0.0)

            # warm-up matmuls read ONLY not-yet-written tiles (garbage values
            # are harmless: results are never read, scores overwrite the
            # PSUM), so the PE starts the moment its preamble ends with no
            # memset dependency, and each read doubles as an issue-gate for
            # the DMA that later writes that tile.
            def warm_mm(w, rhs):
                wp = psA.tile([128, 2, QC], f32, tag="s", name=f"wp{w}")
                nc.tensor.matmul(wp[:, 0, 0:rhs.free_size()],
                                 xab[:, 15, 0:128], rhs,
                                 start=True, stop=True)

            xTc = [xtp.tile([128, 2, QC], f16, tag="xT", name=f"xTc{c}")
                   for c in range(2)]
            nc.sync.dma_start(xTc[0][:], xt_d[0])
            warm_mm(0, xab[:, 0, :])
            nc.scalar.dma_start(wpk[:], wp_d)
            warm_mm(1, xab[:, 0, :])
            warm_mm(2, xab[:, 1, :])
            warm_mm(3, xTc[1][:, 0, :].bitcast(bf16))
            nc.sync.dma_start(xTc[1][:], xt_d[1])
            for w in range(4, 8):
                warm_mm(w, xab[:, w - 2, :])
            nc.gpsimd.dma_start(xab[:], xa_d.rearrange("p (t d) -> p t d", d=DA))
            for c in range(2, NCH):
                t = xtp.tile([128, 2, QC], f16, tag="xT", name=f"xTc{c}")
                nc.sync.dma_start(t[:], xt_d[c])
                xTc.append(t)

            # ---- helpers -------------------------------------------------
            qT = cons.tile([F, S], f32r, tag="qT")
            kT = cons.tile([F, S], f32r, tag="kT")

            def scores_pair(c, PT, p):
                """S^T[k-pair p, q-chunk c] -> exp(. - SHIFT) -> PT (bf16)."""
                sl = slice(c * QC, (c + 1) * QC)
                sp = psA.tile([128, 2, QC], f32, tag="s")
                for j in range(2):
                    kt = 2 * p + j
                    nc.tensor.matmul(sp[:, j, :],
                                     kT[:, kt * 128:(kt + 1) * 128],
                                     qT[:, sl], start=True, stop=True)
                nc.scalar.activation(PT[:, 2 * p:2 * p + 2, :], sp[:],
                                     Exp, bias=biasC[:])

            outbuf = cons.tile([128, NKT, D], f32, tag="outbuf")
            ot_d = out_d.rearrange("(t p) d -> t p d", p=128)

            def out_tile(qt, PT):
                """O_aug[q-tile qt] = sum_k PT_k^T @ x_aug_k ; normalize."""
                qq = qt % 4
                op = psB.tile([128, QC], f32, tag="ot")
                for kt in range(NKT):
                    nc.tensor.matmul(op[:, 0:DA],
                                     PT[:, kt, qq * 128:(qq + 1) * 128],
                                     x_aug[kt],
                                     start=(kt == 0), stop=(kt == NKT - 1))
                rec = scl.tile([128, 1], f32, tag="rec")
                nc.vector.reciprocal(rec[:], op[:, D:D + 1])
                nc.vector.tensor_scalar_mul(outbuf[:, qt, :], op[:, 0:D], rec[:])
                # all output DMAs on the (otherwise idle) sync queue: the
                # teardown drains per-queue, so keeping gpsimd's queue empty
                # late shortens the final drain chain
                nc.sync.dma_start(ot_d[qt], outbuf[:, qt, :])

            # ---- projections (fp16) + relu on DVE + chunk-0 scores -------
            # k first (its relu overlaps the q matmuls, shortening the path
            # to the first score pair); q/k in separate 1-bank PSUM tiles so
            # each relu waits only its own two matmuls. chunk-0 score pairs
            # interleave with the projections so the exp chain (the ACT
            # pacer) starts as early as possible.
            PT0 = ptp.tile([128, NKT, QC], bf16, tag="PT")
            for c in range(NCH):
                sl = slice(c * QC, (c + 1) * QC)
                pk = psB.tile([128, QC], f32, tag="ot")
                for h in range(2):
                    nc.tensor.matmul(pk[:], wk[h], xTc[c][:, h, :],
                                     start=(h == 0), stop=(h == 1))
                nc.vector.tensor_scalar(kT[:, sl], pk[:], bk_t, 0.0, Add, Max)
                pq = psB.tile([128, QC], f32, tag="ot")
                for h in range(2):
                    nc.tensor.matmul(pq[:], wq[h], xTc[c][:, h, :],
                                     start=(h == 0), stop=(h == 1))
                nc.vector.tensor_scalar(qT[:, sl], pq[:], bq_t, 0.0, Add, Max)
                # pairs lag the projections by one chunk so they never wait
                # on the relu that just produced their kT tiles
                if c > 0:
                    scores_pair(0, PT0, 2 * (c - 1))
                    scores_pair(0, PT0, 2 * (c - 1) + 1)
            for p in range(6, 8):
                scores_pair(0, PT0, p)

            # main loop: score pairs front-loaded 3-deep (the psA window) so
            # the exp chain runs gapless while out-tiles fill the PE
            prev = PT0
            for c in range(1, NCH):
                PT = ptp.tile([128, NKT, QC], bf16, tag="PT")
                for p in range(3):
                    scores_pair(c, PT, p)
                out_tile((c - 1) * 4 + 0, prev)
                for p in range(3, 5):
                    scores_pair(c, PT, p)
                out_tile((c - 1) * 4 + 1, prev)
                for p in range(5, 7):
                    scores_pair(c, PT, p)
                out_tile((c - 1) * 4 + 2, prev)
                scores_pair(c, PT, 7)
                out_tile((c - 1) * 4 + 3, prev)
                prev = PT
            for blk in range(4):
                out_tile(3 * 4 + blk, prev)

    nc.compile()
    return nc


def prepare_in_maps(inputs):
    import ml_dtypes
    x = np.ascontiguousarray(inputs["inputs"], dtype=np.float32)
    # fp16 transposed copy for the projections, packed [B, NCH, p, h, s] so
    # each chunk is one contiguous transfer with 2KB per-partition lines
    xt = np.ascontiguousarray(
        x.transpose(0, 2, 1).astype(np.float16)          # [B, 256, 2048]
         .reshape(B, 2, 128, NCH, QC).transpose(0, 3, 2, 1, 4))
    # augmented bf16 copy for the output matmul, packed partition-major:
    # [B, 2048, 258] -> [B, 128, 16*258]
    pad = np.zeros((B, S, DA - D), dtype=np.float32)
    pad[:, :, 0] = 1.0
    xa = np.concatenate([x, pad], axis=2).astype(ml_dtypes.bfloat16)
    xa = np.ascontiguousarray(
        xa.reshape(B, NKT, 128, DA).transpose(0, 2, 1, 3).reshape(B, 128, NKT * DA))
    wq = np.asarray(inputs["Wq"]).astype(np.float16)
    wk = np.asarray(inputs["Wk"]).astype(np.float16)
    bqv = np.asarray(inputs["bq"]).astype(np.float32).reshape(F, 1).view(np.float16)
    bkv = np.asarray(inputs["bk"]).astype(np.float32).reshape(F, 1).view(np.float16)
    wp = np.ascontiguousarray(np.concatenate(
        [wq[:128], wq[128:], wk[:128], wk[128:], bqv, bkv], axis=1))
    return [
        {"xt": xt[b], "xa": xa[b], "wp": wp} for b in range(B)
    ]


def kernel(**inputs):
    _ntff_hook_shim()
    from concourse.bass_utils import run_bass_kernel_spmd

    if "nc" not in _cache:
        _cache["nc"] = _build()
    nc = _cache["nc"]

    in_maps = prepare_in_maps(inputs)
    res = run_bass_kernel_spmd(nc, in_maps, core_ids=list(range(B)))
    out = np.stack([res.results[b]["out"] for b in range(B)], axis=0)
    _cache["last_exec_time_ns"] = res.exec_time_ns
    return out.astype(np.float32)


# revision 47
# speedup vs baseline: 1.0461x; 1.0461x over previous
"""Trainium2 Bass kernel for AttentionLayer: out = softmax(relu(xWq+bq) @ relu(xWk+bk)^T) @ x.

Sharding: data-parallel over batch B=8 across the 8 NeuronCores; Q/K weights
replicated. Each core computes one full [2048, 256] attention independently.

Per-core algorithm (S=2048, D=256, F=128), ~65.5us HW (baseline was 75.6):
  - Host supplies BOTH layouts of x, so the device does zero transposes:
      xt [NCH, 128, 2, 512] fp16 -> Q/K projections (contract over d); the
        [p][h][s] chunk packing makes every partition line one contiguous
        2KB DMA packet (DMA cost is per-packet, not per-byte)
      xa [128, 16*258] bf16 -> x tiles [128, 258] with a [1.0, 0.0] column
        pad (the ones column yields softmax row sums through the output
        matmul for free); 8KB partition lines
  - DMA engines round-robin packets across ALL pending transfers (no FIFO
    between transfers), so arrival order is enforced by gating each issue:
    warm-up junk matmuls "read" the destination tiles first (WAR), and the
    2-buffer xT pool holds chunks c2/c3 behind the projections.
  - qT/kT = relu(W^T @ xT + b) in [f=128, s=2048] layout (fp16 matmuls with
    FWL; relu+bias on DVE via tensor_scalar(add,max) so ACT does exp only).
  - S^T[k, q] = kT^T @ qT per 512-wide q chunk (fp32r, N=512); softmax uses
    a fixed shift exp(s - 60) (scores lie in [2, 94]) written straight to
    bf16 by ACT (a tiny dummy activation up front pre-loads the exp table).
  - O_aug[q, 0:258] = sum_k P^T[:,q]^T @ x_aug[k] in bf16 (FWL hides the
    128-col LDWEIGHTS under the N=258 matmul: 112ns/MM sustained);
    O = O_aug[:, :256] * (1 / O_aug[:, 256]).
  - PE program interleaves scores(c) pairs (front-loaded 3-deep = the psA
    window) with out(c-1) q-tiles; Tile's scheduler then trickles out-tile
    matmuls per-k as each exp lands, keeping the PE gapless start to end.
  - fp16 proj path + bf16 P/V keep error ~4.6e-3 of absmax (budget 2e-2).
"""

import sys
import types
from contextlib import ExitStack

import numpy as np

B, S, D, F = 8, 2048, 256, 128
DA = D + 2           # x padded with [ones, zero] columns
SHIFT = 60.0          # fixed softmax shift; scores lie in [2, 94]
QC = 512              # q-chunk width for the scores/exp/output pipeline
NKT = S // 128        # 16 sequence tiles
NCH = S // QC         # 4 q chunks

_cache = {}


def _ntff_hook_shim():
    """The image's antenv lacks axon_hooks; reconstruct the NTFF profile hook
    so run_bass_kernel_spmd(trace=True) works. Harmless if it fails."""
    if "antenv.axon_hooks" in sys.modules:
        return
    try:
        from trn_agent_boot.trn_boot import _ntff_profile_via_ctypes
        hook = _ntff_profile_via_ctypes("/opt/axon/libaxon_pjrt.so")
        mod = types.ModuleType("antenv.axon_hooks")
        mod.get_axon_ntff_profile_hook = lambda: hook
        mod.set_axon_ntff_profile_hook = lambda h: None
        sys.modules["antenv.axon_hooks"] = mod
    except Exception:
        pass


def _build():
    import concourse.bacc as bacc
    import concourse.tile as tile
    from concourse import mybir

    f32 = mybir.dt.float32
    f32r = mybir.dt.float32r
    f16 = mybir.dt.float16
    bf16 = mybir.dt.bfloat16
    Exp = mybir.ActivationFunctionType.Exp
    Add = mybir.AluOpType.add
    Max = mybir.AluOpType.max

    nc = bacc.Bacc("TRN2", target_bir_lowering=False, debug=False)
    xt_d = nc.dram_tensor("xt", [NCH, 128, 2, QC], f16, kind="ExternalInput").ap()
    xa_d = nc.dram_tensor("xa", [128, NKT * DA], bf16, kind="ExternalInput").ap()
    # wq halves | wk halves | bq | bk packed as one [128, 4*128+4] fp16 blob
    # (biases are fp32 pairs bitcast into the last 4 fp16 columns) so the
    # whole weight load is one DMA with 1KB contiguous partition lines
    wp_d = nc.dram_tensor("wp", [128, 4 * F + 4], f16, kind="ExternalInput").ap()
    out_d = nc.dram_tensor("out", [S, D], f32, kind="ExternalOutput").ap()

    with tile.TileContext(nc) as tc:
        with ExitStack() as ctx:
            cons = ctx.enter_context(tc.tile_pool(name="cons", bufs=1))
            ptp = ctx.enter_context(tc.tile_pool(name="ptp", bufs=2))
            scl = ctx.enter_context(tc.tile_pool(name="scl", bufs=4))
            psA = ctx.enter_context(tc.tile_pool(name="psA", bufs=3, space="PSUM"))
            psB = ctx.enter_context(tc.tile_pool(name="psB", bufs=2, space="PSUM"))

            # ---- input DMAs + PE warm-up, interlocked --------------------
            # The DMA engines round-robin packets across every pending
            # transfer (no FIFO priority between transfers), so concurrent
            # transfers all finish late together. To stream the xT chunks in
            # arrival order, each transfer's issue is GATED on a warm-up junk
            # matmul that "reads" the destination tile before the DMA writes
            # it (WAR dependency): dma(c1) waits w1, dma(xa) waits w2, and
            # the chunk pool's 2 buffers gate c2/c3 on the projections.
            # The junk matmuls double as the HAM warm-up (~0.43us each cold).
            # Garbage operand values are fine: warm-up results are never
            # read and later scores overwrite the PSUM banks.
            xtp = ctx.enter_context(tc.tile_pool(name="xtp", bufs=2))
            junk = cons.tile([128, QC], bf16, tag="junk")
            nc.gpsimd.memset(junk[:], 0.0)
            wpk = cons.tile([128, 4 * F + 4], f16, tag="wpk")
            nc.scalar.dma_start(wpk[:], wp_d)
            wq = [wpk[:, h * F:(h + 1) * F] for h in range(2)]
            wk = [wpk[:, (2 + h) * F:(3 + h) * F] for h in range(2)]
            bq_t = wpk[:, 4 * F:4 * F + 2].bitcast(f32)
            bk_t = wpk[:, 4 * F + 2:4 * F + 4].bitcast(f32)
            biasC = cons.tile([128, 1], f32, tag="biasC")
            nc.gpsimd.memset(biasC[:], -SHIFT)
            xab = cons.tile([128, NKT, DA], bf16, tag="xab")
            x_aug = [xab[:, kt, :] for kt in range(NKT)]

            # dummy activation right away: pulls ACT_TABLE_LOAD (1.3us) off
            # the first-exp critical path (it otherwise waits on the first
            # score matmuls through the scheduler's ordering)
            dum = cons.tile([128, 2], f32, tag="dum")
            nc.scalar.activation(dum[:], junk[:, 0:2], Exp, bias=0.0)

            def warm_mm(w, rhs):
                wp = psA.tile([128, 2, QC], f32, tag="s", name=f"wp{w}")
                nc.tensor.matmul(wp[:, 0, 0:rhs.free_size()], junk[:, 0:128],
                                 rhs, start=True, stop=True)

            xTc = [xtp.tile([128, 2, QC], f16, tag="xT", name=f"xTc{c}")
                   for c in range(2)]
            nc.sync.dma_start(xTc[0][:], xt_d[0])
            warm_mm(0, junk[:])
            warm_mm(1, xTc[1][:, 0, :].bitcast(bf16))
            nc.sync.dma_start(xTc[1][:], xt_d[1])
            warm_mm(2, xab[:, 0, :])
            nc.gpsimd.dma_start(xab[:], xa_d.rearrange("p (t d) -> p t d", d=DA))
            for w in range(3, 8):
                warm_mm(w, junk[:])
            for c in range(2, NCH):
                t = xtp.tile([128, 2, QC], f16, tag="xT", name=f"xTc{c}")
                nc.sync.dma_start(t[:], xt_d[c])
                xTc.append(t)

            # ---- helpers -------------------------------------------------
            qT = cons.tile([F, S], f32r, tag="qT")
            kT = cons.tile([F, S], f32r, tag="kT")

            def scores_pair(c, PT, p):
                """S^T[k-pair p, q-chunk c] -> exp(. - SHIFT) -> PT (bf16)."""
                sl = slice(c * QC, (c + 1) * QC)
                sp = psA.tile([128, 2, QC], f32, tag="s")
                for j in range(2):
                    kt = 2 * p + j
                    nc.tensor.matmul(sp[:, j, :],
                                     kT[:, kt * 128:(kt + 1) * 128],
                                     qT[:, sl], start=True, stop=True)
                nc.scalar.activation(PT[:, 2 * p:2 * p + 2, :], sp[:],
                                     Exp, bias=biasC[:])

            outbuf = cons.tile([128, NKT, D], f32, tag="outbuf")
            ot_d = out_d.rearrange("(t p) d -> t p d", p=128)

            def out_tile(qt, PT):
                """O_aug[q-tile qt] = sum_k PT_k^T @ x_aug_k ; normalize."""
                qq = qt % 4
                op = psB.tile([128, QC], f32, tag="ot")
                for kt in range(NKT):
                    nc.tensor.matmul(op[:, 0:DA],
                                     PT[:, kt, qq * 128:(qq + 1) * 128],
                                     x_aug[kt],
                                     start=(kt == 0), stop=(kt == NKT - 1))
                rec = scl.tile([128, 1], f32, tag="rec")
                nc.vector.reciprocal(rec[:], op[:, D:D + 1])
                nc.vector.tensor_scalar_mul(outbuf[:, qt, :], op[:, 0:D], rec[:])
                # all output DMAs on the (otherwise idle) sync queue: the
                # teardown drains per-queue, so keeping gpsimd's queue empty
                # late shortens the final drain chain
                nc.sync.dma_start(ot_d[qt], outbuf[:, qt, :])

            # ---- projections (fp16) + relu on DVE + chunk-0 scores -------
            # k first (its relu overlaps the q matmuls, shortening the path
            # to the first score pair); q/k in separate 1-bank PSUM tiles so
            # each relu waits only its own two matmuls. chunk-0 score pairs
            # interleave with the projections so the exp chain (the ACT
            # pacer) starts as early as possible.
            PT0 = ptp.tile([128, NKT, QC], bf16, tag="PT")
            for c in range(NCH):
                sl = slice(c * QC, (c + 1) * QC)
                pk = psB.tile([128, QC], f32, tag="ot")
                for h in range(2):
                    nc.tensor.matmul(pk[:], wk[h], xTc[c][:, h, :],
                                     start=(h == 0), stop=(h == 1))
                nc.vector.tensor_scalar(kT[:, sl], pk[:], bk_t, 0.0, Add, Max)
                pq = psB.tile([128, QC], f32, tag="ot")
                for h in range(2):
                    nc.tensor.matmul(pq[:], wq[h], xTc[c][:, h, :],
                                     start=(h == 0), stop=(h == 1))
                nc.vector.tensor_scalar(qT[:, sl], pq[:], bq_t, 0.0, Add, Max)
                # pairs lag the projections by one chunk so they never wait
                # on the relu that just produced their kT tiles
                if c > 0:
                    scores_pair(0, PT0, 2 * (c - 1))
                    scores_pair(0, PT0, 2 * (c - 1) + 1)
            for p in range(6, 8):
                scores_pair(0, PT0, p)

            # main loop: score pairs front-loaded 3-deep (the psA window) so
            # the exp chain runs gapless while out-tiles fill the PE
            prev = PT0
            for c in range(1, NCH):
                PT = ptp.tile([128, NKT, QC], bf16, tag="PT")
                for p in range(3):
                    scores_pair(c, PT, p)
                out_tile((c - 1) * 4 + 0, prev)
                for p in range(3, 5):
                    scores_pair(c, PT, p)
                out_tile((c - 1) * 4 + 1, prev)
                for p in range(5, 7):
                    scores_pair(c, PT, p)
                out_tile((c - 1) * 4 + 2, prev)
                scores_pair(c, PT, 7)
                out_tile((c - 1) * 4 + 3, prev)
                prev = PT
            for blk in range(4):
                out_tile(3 * 4 + blk, prev)

    nc.compile()
    return nc


def prepare_in_maps(inputs):
    import ml_dtypes
    x = np.ascontiguousarray(inputs["inputs"], dtype=np.float32)
    # fp16 transposed copy for the projections, packed [B, NCH, p, h, s] so
    # each chunk is one contiguous transfer with 2KB per-partition lines
    xt = np.ascontiguousarray(
        x.transpose(0, 2, 1).astype(np.float16)          # [B, 256, 2048]
         .reshape(B, 2, 128, NCH, QC).transpose(0, 3, 2, 1, 4))
    # augmented bf16 copy for the output matmul, packed partition-major:
    # [B, 2048, 258] -> [B, 128, 16*258]
    pad = np.zeros((B, S, DA - D), dtype=np.float32)
    pad[:, :, 0] = 1.0
    xa = np.concatenate([x, pad], axis=2).astype(ml_dtypes.bfloat16)
    xa = np.ascontiguousarray(
        xa.reshape(B, NKT, 128, DA).transpose(0, 2, 1, 3).reshape(B, 128, NKT * DA))
    wq = np.asarray(inputs["Wq"]).astype(np.float16)
    wk = np.asarray(inputs["Wk"]).astype(np.float16)
    bqv = np.asarray(inputs["bq"]).astype(np.float32).reshape(F, 1).view(np.float16)
    bkv = np.asarray(inputs["bk"]).astype(np.float32).reshape(F, 1).view(np.float16)
    wp = np.ascontiguousarray(np.concatenate(
        [wq[:128], wq[128:], wk[:128], wk[128:], bqv, bkv], axis=1))
    return [
        {"xt": xt[b], "xa": xa[b], "wp": wp} for b in range(B)
    ]


def kernel(**inputs):
    _ntff_hook_shim()
    from concourse.bass_utils import run_bass_kernel_spmd

    if "nc" not in _cache:
        _cache["nc"] = _build()
    nc = _cache["nc"]

    in_maps = prepare_in_maps(inputs)
    res = run_bass_kernel_spmd(nc, in_maps, core_ids=list(range(B)))
    out = np.stack([res.results[b]["out"] for b in range(B)], axis=0)
    _cache["last_exec_time_ns"] = res.exec_time_ns
    return out.astype(np.float32)


# revision 48
# speedup vs baseline: 1.0632x; 1.0163x over previous
"""Trainium2 Bass kernel for AttentionLayer: out = softmax(relu(xWq+bq) @ relu(xWk+bk)^T) @ x.

Sharding: data-parallel over batch B=8 across the 8 NeuronCores; Q/K weights
replicated. Each core computes one full [2048, 256] attention independently.

Per-core algorithm (S=2048, D=256, F=128), ~65.5us HW (baseline was 75.6):
  - Host supplies BOTH layouts of x, so the device does zero transposes:
      xt [NCH, 128, 2, 512] fp16 -> Q/K projections (contract over d); the
        [p][h][s] chunk packing makes every partition line one contiguous
        2KB DMA packet (DMA cost is per-packet, not per-byte)
      xa [128, 16*258] bf16 -> x tiles [128, 258] with a [1.0, 0.0] column
        pad (the ones column yields softmax row sums through the output
        matmul for free); 8KB partition lines
  - DMA engines round-robin packets across ALL pending transfers (no FIFO
    between transfers), so arrival order is enforced by gating each issue:
    warm-up junk matmuls "read" the destination tiles first (WAR), and the
    2-buffer xT pool holds chunks c2/c3 behind the projections.
  - qT/kT = relu(W^T @ xT + b) in [f=128, s=2048] layout (fp16 matmuls with
    FWL; relu+bias on DVE via tensor_scalar(add,max) so ACT does exp only).
  - S^T[k, q] = kT^T @ qT per 512-wide q chunk (fp32r, N=512); softmax uses
    a fixed shift exp(s - 60) (scores lie in [2, 94]) written straight to
    bf16 by ACT (a tiny dummy activation up front pre-loads the exp table).
  - O_aug[q, 0:258] = sum_k P^T[:,q]^T @ x_aug[k] in bf16 (FWL hides the
    128-col LDWEIGHTS under the N=258 matmul: 112ns/MM sustained);
    O = O_aug[:, :256] * (1 / O_aug[:, 256]).
  - PE program interleaves scores(c) pairs (front-loaded 3-deep = the psA
    window) with out(c-1) q-tiles; Tile's scheduler then trickles out-tile
    matmuls per-k as each exp lands, keeping the PE gapless start to end.
  - fp16 proj path + bf16 P/V keep error ~4.6e-3 of absmax (budget 2e-2).
"""

import sys
import types
from contextlib import ExitStack

import numpy as np

B, S, D, F = 8, 2048, 256, 128
DA = D + 2           # x padded with [ones, zero] columns
SHIFT = 60.0          # fixed softmax shift; scores lie in [2, 94]
QC = 512              # q-chunk width for the scores/exp/output pipeline
NKT = S // 128        # 16 sequence tiles
NCH = S // QC         # 4 q chunks

_cache = {}


def _ntff_hook_shim():
    """The image's antenv lacks axon_hooks; reconstruct the NTFF profile hook
    so run_bass_kernel_spmd(trace=True) works. Harmless if it fails."""
    if "antenv.axon_hooks" in sys.modules:
        return
    try:
        from trn_agent_boot.trn_boot import _ntff_profile_via_ctypes
        hook = _ntff_profile_via_ctypes("/opt/axon/libaxon_pjrt.so")
        mod = types.ModuleType("antenv.axon_hooks")
        mod.get_axon_ntff_profile_hook = lambda: hook
        mod.set_axon_ntff_profile_hook = lambda h: None
        sys.modules["antenv.axon_hooks"] = mod
    except Exception:
        pass


def _build():
    import concourse.bacc as bacc
    import concourse.tile as tile
    from concourse import mybir

    f32 = mybir.dt.float32
    f32r = mybir.dt.float32r
    f16 = mybir.dt.float16
    bf16 = mybir.dt.bfloat16
    Exp = mybir.ActivationFunctionType.Exp
    Add = mybir.AluOpType.add
    Max = mybir.AluOpType.max

    nc = bacc.Bacc("TRN2", target_bir_lowering=False, debug=False)
    xt_d = nc.dram_tensor("xt", [NCH, 128, 2, QC], f16, kind="ExternalInput").ap()
    xa_d = nc.dram_tensor("xa", [128, NKT * DA], bf16, kind="ExternalInput").ap()
    # wq halves | wk halves | bq | bk packed as one [128, 4*128+4] fp16 blob
    # (biases are fp32 pairs bitcast into the last 4 fp16 columns) so the
    # whole weight load is one DMA with 1KB contiguous partition lines
    wp_d = nc.dram_tensor("wp", [128, 4 * F + 4], f16, kind="ExternalInput").ap()
    out_d = nc.dram_tensor("out", [S, D], f32, kind="ExternalOutput").ap()

    with tile.TileContext(nc) as tc:
        with ExitStack() as ctx:
            cons = ctx.enter_context(tc.tile_pool(name="cons", bufs=1))
            ptp = ctx.enter_context(tc.tile_pool(name="ptp", bufs=2))
            scl = ctx.enter_context(tc.tile_pool(name="scl", bufs=4))
            psA = ctx.enter_context(tc.tile_pool(name="psA", bufs=3, space="PSUM"))
            psB = ctx.enter_context(tc.tile_pool(name="psB", bufs=2, space="PSUM"))

            # ---- input DMAs + PE warm-up, interlocked --------------------
            # The DMA engines round-robin packets across every pending
            # transfer (no FIFO priority between transfers), so concurrent
            # transfers all finish late together. To stream the xT chunks in
            # arrival order, each transfer's issue is GATED on a warm-up junk
            # matmul that "reads" the destination tile before the DMA writes
            # it (WAR dependency): dma(c1) waits w1, dma(xa) waits w2, and
            # the chunk pool's 2 buffers gate c2/c3 on the projections.
            # The junk matmuls double as the HAM warm-up (~0.43us each cold).
            # Garbage operand values are fine: warm-up results are never
            # read and later scores overwrite the PSUM banks.
            xtp = ctx.enter_context(tc.tile_pool(name="xtp", bufs=2))
            junk = cons.tile([128, QC], bf16, tag="junk")
            nc.gpsimd.memset(junk[:], 0.0)
            wpk = cons.tile([128, 4 * F + 4], f16, tag="wpk")
            nc.scalar.dma_start(wpk[:], wp_d)
            wq = [wpk[:, h * F:(h + 1) * F] for h in range(2)]
            wk = [wpk[:, (2 + h) * F:(3 + h) * F] for h in range(2)]
            bq_t = wpk[:, 4 * F:4 * F + 2].bitcast(f32)
            bk_t = wpk[:, 4 * F + 2:4 * F + 4].bitcast(f32)
            biasC = cons.tile([128, 1], f32, tag="biasC")
            nc.gpsimd.memset(biasC[:], -SHIFT)
            xab = cons.tile([128, NKT, DA], bf16, tag="xab")
            x_aug = [xab[:, kt, :] for kt in range(NKT)]

            # dummy activation right away: pulls ACT_TABLE_LOAD (1.3us) off
            # the first-exp critical path (it otherwise waits on the first
            # score matmuls through the scheduler's ordering)
            dum = cons.tile([128, 2], f32, tag="dum")
            nc.scalar.activation(dum[:], junk[:, 0:2], Exp, bias=0.0)

            def warm_mm(w, rhs):
                wp = psA.tile([128, 2, QC], f32, tag="s", name=f"wp{w}")
                nc.tensor.matmul(wp[:, 0, 0:rhs.free_size()], junk[:, 0:128],
                                 rhs, start=True, stop=True)

            xTc = [xtp.tile([128, 2, QC], f16, tag="xT", name=f"xTc{c}")
                   for c in range(2)]
            nc.sync.dma_start(xTc[0][:], xt_d[0])
            warm_mm(0, junk[:])
            warm_mm(1, xTc[1][:, 0, :].bitcast(bf16))
            nc.sync.dma_start(xTc[1][:], xt_d[1])
            warm_mm(2, xab[:, 0, :])
            nc.gpsimd.dma_start(xab[:], xa_d.rearrange("p (t d) -> p t d", d=DA))
            for w in range(3, 8):
                warm_mm(w, junk[:])
            for c in range(2, NCH):
                t = xtp.tile([128, 2, QC], f16, tag="xT", name=f"xTc{c}")
                nc.sync.dma_start(t[:], xt_d[c])
                xTc.append(t)

            # ---- helpers -------------------------------------------------
            # qT/kT in fp16: the scores matmuls then get fast weight load
            # (LDWEIGHTS ~80ns instead of 185), which matters at the 32
            # score-pair boundaries where the LDW is not fully hidden
            qT = cons.tile([F, S], f16, tag="qT")
            kT = cons.tile([F, S], f16, tag="kT")

            def scores_pair(c, PT, p):
                """S^T[k-pair p, q-chunk c] -> exp(. - SHIFT) -> PT (bf16)."""
                sl = slice(c * QC, (c + 1) * QC)
                sp = psA.tile([128, 2, QC], f32, tag="s")
                for j in range(2):
                    kt = 2 * p + j
                    nc.tensor.matmul(sp[:, j, :],
                                     kT[:, kt * 128:(kt + 1) * 128],
                                     qT[:, sl], start=True, stop=True)
                nc.scalar.activation(PT[:, 2 * p:2 * p + 2, :], sp[:],
                                     Exp, bias=biasC[:])

            outbuf = cons.tile([128, NKT, D], f32, tag="outbuf")
            ot_d = out_d.rearrange("(t p) d -> t p d", p=128)

            def out_tile(qt, PT):
                """O_aug[q-tile qt] = sum_k PT_k^T @ x_aug_k ; normalize."""
                qq = qt % 4
                op = psB.tile([128, QC], f32, tag="ot")
                for kt in range(NKT):
                    nc.tensor.matmul(op[:, 0:DA],
                                     PT[:, kt, qq * 128:(qq + 1) * 128],
                                     x_aug[kt],
                                     start=(kt == 0), stop=(kt == NKT - 1))
                rec = scl.tile([128, 1], f32, tag="rec")
                nc.vector.reciprocal(rec[:], op[:, D:D + 1])
                nc.vector.tensor_scalar_mul(outbuf[:, qt, :], op[:, 0:D], rec[:])
                # all output DMAs on the (otherwise idle) sync queue: the
                # teardown drains per-queue, so keeping gpsimd's queue empty
                # late shortens the final drain chain
                nc.sync.dma_start(ot_d[qt], outbuf[:, qt, :])

            # ---- projections (fp16) + relu on DVE + chunk-0 scores -------
            # k first (its relu overlaps the q matmuls, shortening the path
            # to the first score pair); q/k in separate 1-bank PSUM tiles so
            # each relu waits only its own two matmuls. chunk-0 score pairs
            # interleave with the projections so the exp chain (the ACT
            # pacer) starts as early as possible.
            PT0 = ptp.tile([128, NKT, QC], bf16, tag="PT")
            for c in range(NCH):
                sl = slice(c * QC, (c + 1) * QC)
                pk = psB.tile([128, QC], f32, tag="ot")
                for h in range(2):
                    nc.tensor.matmul(pk[:], wk[h], xTc[c][:, h, :],
                                     start=(h == 0), stop=(h == 1))
                nc.vector.tensor_scalar(kT[:, sl], pk[:], bk_t, 0.0, Add, Max)
                pq = psB.tile([128, QC], f32, tag="ot")
                for h in range(2):
                    nc.tensor.matmul(pq[:], wq[h], xTc[c][:, h, :],
                                     start=(h == 0), stop=(h == 1))
                nc.vector.tensor_scalar(qT[:, sl], pq[:], bq_t, 0.0, Add, Max)
                # pairs lag the projections by one chunk so they never wait
                # on the relu that just produced their kT tiles
                if c > 0:
                    scores_pair(0, PT0, 2 * (c - 1))
                    scores_pair(0, PT0, 2 * (c - 1) + 1)
            for p in range(6, 8):
                scores_pair(0, PT0, p)

            # main loop: score pairs front-loaded 3-deep (the psA window) so
            # the exp chain runs gapless while out-tiles fill the PE
            prev = PT0
            for c in range(1, NCH):
                PT = ptp.tile([128, NKT, QC], bf16, tag="PT")
                for p in range(3):
                    scores_pair(c, PT, p)
                out_tile((c - 1) * 4 + 0, prev)
                for p in range(3, 5):
                    scores_pair(c, PT, p)
                out_tile((c - 1) * 4 + 1, prev)
                for p in range(5, 7):
                    scores_pair(c, PT, p)
                out_tile((c - 1) * 4 + 2, prev)
                scores_pair(c, PT, 7)
                out_tile((c - 1) * 4 + 3, prev)
                prev = PT
            for blk in range(4):
                out_tile(3 * 4 + blk, prev)

    nc.compile()
    return nc


def prepare_in_maps(inputs):
    import ml_dtypes
    x = np.ascontiguousarray(inputs["inputs"], dtype=np.float32)
    # fp16 transposed copy for the projections, packed [B, NCH, p, h, s] so
    # each chunk is one contiguous transfer with 2KB per-partition lines
    xt = np.ascontiguousarray(
        x.transpose(0, 2, 1).astype(np.float16)          # [B, 256, 2048]
         .reshape(B, 2, 128, NCH, QC).transpose(0, 3, 2, 1, 4))
    # augmented bf16 copy for the output matmul, packed partition-major:
    # [B, 2048, 258] -> [B, 128, 16*258]
    pad = np.zeros((B, S, DA - D), dtype=np.float32)
    pad[:, :, 0] = 1.0
    xa = np.concatenate([x, pad], axis=2).astype(ml_dtypes.bfloat16)
    xa = np.ascontiguousarray(
        xa.reshape(B, NKT, 128, DA).transpose(0, 2, 1, 3).reshape(B, 128, NKT * DA))
    wq = np.asarray(inputs["Wq"]).astype(np.float16)
    wk = np.asarray(inputs["Wk"]).astype(np.float16)
    bqv = np.asarray(inputs["bq"]).astype(np.float32).reshape(F, 1).view(np.float16)
    bkv = np.asarray(inputs["bk"]).astype(np.float32).reshape(F, 1).view(np.float16)
    wp = np.ascontiguousarray(np.concatenate(
        [wq[:128], wq[128:], wk[:128], wk[128:], bqv, bkv], axis=1))
    return [
        {"xt": xt[b], "xa": xa[b], "wp": wp} for b in range(B)
    ]


def kernel(**inputs):
    _ntff_hook_shim()
    from concourse.bass_utils import run_bass_kernel_spmd

    if "nc" not in _cache:
        _cache["nc"] = _build()
    nc = _cache["nc"]

    in_maps = prepare_in_maps(inputs)
    res = run_bass_kernel_spmd(nc, in_maps, core_ids=list(range(B)))
    out = np.stack([res.results[b]["out"] for b in range(B)], axis=0)
    _cache["last_exec_time_ns"] = res.exec_time_ns
    return out.astype(np.float32)
